# revision 55
# baseline (speedup 1.0000x reference)
"""Grouped cross-attention Trainium2 kernel (106.0us vs 201.6us baseline).

Problem: B=4, SQ=1024, SK=2048, D=1024, H=16 heads (HD=64), G=4 groups
(GD=256) grouped o_proj, key/query masks, softmax over keys.

Sharding: 8 cores = (batch b = c//2) x (half of heads s = c%2).
Each core computes attention for 8 heads (= 2 o_proj groups) of one batch
and produces out[b, :, s*512:(s+1)*512] (transposed on device).

Key design points (all hardware-validated):
  * all PE matmuls bf16: streams 1 col/cycle at the 1.2GHz the PE
    actually sustains here (fp32/f32r lowers to multi-pass, ~1.5ns/col;
    2.4GHz never engages on this part, even after 90us continuous load)
  * host-side mask compression: only unmasked rows are shipped; padded
    va rows are zeroed so padding contributes exactly 0 to softmax
    numerator AND denominator -> no key-mask bias, no query-mask mul;
    sqp padded to 4 (not 128) since nothing needs 128-alignment
  * exp ACT instructions merged across kc pairs (one AP spanning 2 PSUM
    banks) to amortize the ~200ns fixed cost; exp table pre-loaded via
    a dummy activation during the startup DMA wait
  * normalize: plain nc.vector.reciprocal (reciprocal_approx_fast
    returns garbage on hw) + gpsimd.partition_broadcast (works, frees
    PE + 2 PSUM banks vs a ones-outer-product matmul)
  * o_proj transposed (out^T[o,q]) so bias-add is one per-partition ACT
    op; emitted after both chunks' head loops to keep the PE dense
  * QK emitted one kc-pair ahead of exp/PV (software pipeline, ps_s
    bufs=3) -> PE idle ~4us total mid-kernel
  * inputs sent in device layout, one chunky contiguous DMA each
    (per-partition rows of 2-9KB; a gather-pattern DMA with ~130B
    descriptors crawls at ~40GB/s); issued from the idle GPSIMD queue
    (36ns/issue vs 565ns on Sync) in exact first-need order — bulk DMA
    runs ~41GB/s effective, so arrival order is what gates the first
    three head-segments

Device dataflow per (q-half c, head h):
  for kc pairs: S^T[k,q] = K_h^T.T @ Q_h^T (PE bf16) -> one exp ACT over
  both banks -> E bf16; O'[65,q] += [V_h|1].T @ E (PE bf16, row 64 =
  softmax denominators). Then rq = 1/O'[64] (DVE), sb = partition_
  broadcast(rq) (GPSIMD), O_norm = O'[0:64] * sb (DVE, bf16 out).
  o_proj: out^T[o,q] = sum W^T chunk @ O_norm + bias (PE + ACT), with
  head-pair outputs stacked into [128, q] tiles via SBUF->SBUF DMA
  (partition-shifting the odd head) so the contraction runs 128 deep.
"""

import numpy as np
import ml_dtypes

import concourse.bass as bass
import concourse.mybir as mybir
import concourse.tile as tile
from concourse import bacc
from concourse.bass_utils import run_bass_kernel_spmd

f32 = mybir.dt.float32
bf16 = mybir.dt.bfloat16
BF = ml_dtypes.bfloat16

B, SQ, SK, D, H, HD, G, GD = 4, 1024, 2048, 1024, 16, 64, 4, 256
NCORE = 8
DS = D // 2          # dims per core (8 heads)
HPC = 8              # heads per core
P = 128

TRACE = False        # test.py sets kernel.TRACE = True for profiling
LAST_RUN = {}        # test.py reads exec_time_ns etc. from here
USE_RECIP_FAST = False

_CACHE = {}


def _pad_up(n, m):
    return ((n + m - 1) // m) * m


def build_nc(sqp, skp):
    """Build the per-core Bass program for padded shapes [sqp, skp]."""
    nkc = skp // P
    npair = (nkc + 1) // 2
    qn = sqp // 2
    assert qn <= 512

    nc = bacc.Bacc("TRN2", target_bir_lowering=False, debug=False,
                   num_devices=NCORE)

    # all inputs pre-laid-out host-side in device order so every DMA
    # partition-row is one large contiguous descriptor
    qt_d = nc.dram_tensor("qt", [P, 4, sqp], bf16, kind="ExternalInput")
    kt_d = nc.dram_tensor("kt", [P, 4, skp], bf16, kind="ExternalInput")
    va_d = nc.dram_tensor("va", [P, nkc, HPC, HD + 1], bf16,
                          kind="ExternalInput")
    wt_d = nc.dram_tensor("wt", [HD, 16, P], bf16, kind="ExternalInput")
    wt2_d = nc.dram_tensor("wt2", [P, 8, P], bf16, kind="ExternalInput")
    bt_d = nc.dram_tensor("bt", [P, 4], f32, kind="ExternalInput")
    out_d = nc.dram_tensor("out", [DS, sqp], f32, kind="ExternalOutput")

    with tile.TileContext(nc) as tc:
        with (
            tc.tile_pool(name="big", bufs=1) as big,
            tc.tile_pool(name="consts", bufs=1) as consts,
            tc.tile_pool(name="e_pool", bufs=3) as e_pool,
            tc.tile_pool(name="on_pool", bufs=2) as on_pool,
            tc.tile_pool(name="small", bufs=4) as small,
            tc.tile_pool(name="fo_pool", bufs=1) as fo_pool,
            tc.tile_pool(name="ps_s_pool", bufs=3, space="PSUM") as ps_s_pool,
            tc.tile_pool(name="ps_o_pool", bufs=2, space="PSUM") as ps_o_pool,
        ):
            # ---- static loads: chunky contiguous DMAs, earliest-needed
            # first (kt/qt j=0 gate the first matmul; va kc 0-4 gate the
            # first PV)
            kt_t = big.tile([P, 4, skp], bf16, name="kt_t")
            qt_t = big.tile([P, 4, sqp], bf16, name="qt_t")
            va_t = big.tile([P, nkc, HPC, HD + 1], bf16, name="va_t")
            wt_t = consts.tile([HD, 16, P], bf16, name="wt_t")
            wt2_t = consts.tile([P, 8, P], bf16, name="wt2_t")
            bt_s = consts.tile([P, 4], f32, name="bt_s")
            # issue from the idle GPSIMD queue (36ns/issue vs 565ns on
            # Sync) in exact need-order: head 0-1 consume kt0/qt0/va
            # immediately; kt_j for heads 2j arrives ~11us/j later
            n1 = min(2, nkc)
            nh = max(n1, (nkc + 1) // 2)
            nc.gpsimd.dma_start(out=kt_t[:, 0, :], in_=kt_d[:, 0, :])
            nc.gpsimd.dma_start(out=qt_t[:, 0, :], in_=qt_d[:, 0, :])
            nc.gpsimd.dma_start(out=va_t[:, :n1], in_=va_d[:, :n1])
            if nh > n1:
                nc.gpsimd.dma_start(out=va_t[:, n1:nh], in_=va_d[:, n1:nh])
            if nkc > nh:
                nc.gpsimd.dma_start(out=va_t[:, nh:], in_=va_d[:, nh:])
            nc.gpsimd.dma_start(out=kt_t[:, 1, :], in_=kt_d[:, 1, :])
            nc.gpsimd.dma_start(out=qt_t[:, 1, :], in_=qt_d[:, 1, :])
            for j in range(2, 4):
                nc.gpsimd.dma_start(out=kt_t[:, j, :], in_=kt_d[:, j, :])
                nc.gpsimd.dma_start(out=qt_t[:, j, :], in_=qt_d[:, j, :])
            nc.gpsimd.dma_start(out=wt_t, in_=wt_d[:, :, :])
            nc.gpsimd.dma_start(out=wt2_t, in_=wt2_d[:, :, :])
            nc.gpsimd.dma_start(out=bt_s, in_=bt_d[:, :])

            # dummy exp during the startup DMA wait so the ACT table load
            # (1.3us) is off the critical path
            wdum = consts.tile([P, 16], f32, name="wdum")
            nc.vector.memset(wdum, 0.0)
            wdum2 = consts.tile([P, 16], bf16, name="wdum2")
            nc.scalar.activation(wdum2[:, :], wdum[:, :],
                                 mybir.ActivationFunctionType.Exp,
                                 bias=0.0, scale=0.125)

            # ---- main loops ----
            fo_s = {}
            for g in range(2):
                for ot in range(2):
                    fo_s[(g, ot)] = fo_pool.tile(
                        [P, sqp], f32, tag=f"fo{g}{ot}", name=f"fo{g}{ot}")
            pairs = []
            for kp in range(npair):
                pairs.append([2 * kp, 2 * kp + 1] if 2 * kp + 1 < nkc
                             else [2 * kp])
            steps = [(c, h, kp) for c in range(2) for h in range(HPC)
                     for kp in range(npair)]
            on_all = {0: [], 1: []}
            on_head = {}
            on2_all = {}
            ps_o_cur = {}
            ps_pend = {}
            ps_acc = {}

            def emit_qk(step):
                c, h, kp = step
                j, off = h // 2, (h % 2) * HD
                q0 = c * qn
                ps_s = ps_s_pool.tile([P, 2, 512], f32, tag="ps_s",
                                      name="ps_s")
                for idx, kc in enumerate(pairs[kp]):
                    nc.tensor.matmul(
                        ps_s[:, idx, :qn],
                        kt_t[off:off + HD, j, kc * P:(kc + 1) * P],
                        qt_t[off:off + HD, j, q0:q0 + qn],
                        start=True, stop=True)
                ps_pend[step] = ps_s

            def emit_oproj(cc):
                # out^T[o, q]; 128-deep contraction over head pairs, ot
                # units interleaved; the (c=1, heads 6/7) pair keeps the
                # 64-deep path so the tail never waits on the stack DMA
                q0c = cc * qn
                for g in range(2):
                    ps_t = {ot: ps_s_pool.tile([P, 2, 512], f32,
                                               tag="ps_s", name="ps_t")
                            for ot in range(2)}
                    for p in range(2):
                        if cc == 1 and g == 1 and p == 1:
                            for hh in (6, 7):
                                for ot in range(2):
                                    nc.tensor.matmul(
                                        ps_t[ot][:, 0, :qn],
                                        wt_t[:, (2 * g + ot) * 4 + hh - 4, :],
                                        on_head[(cc, hh)][:, :qn],
                                        start=False, stop=(hh == 7))
                        else:
                            for ot in range(2):
                                nc.tensor.matmul(
                                    ps_t[ot][:, 0, :qn],
                                    wt2_t[:, g * 4 + ot * 2 + p, :],
                                    on2_all[(cc, 2 * g + p)][:, :qn],
                                    start=(p == 0),
                                    stop=(p == 1 and not (cc == 1 and g == 1)))
                    for ot in range(2):
                        nc.scalar.activation(
                            fo_s[(g, ot)][:, q0c:q0c + qn],
                            ps_t[ot][:, 0, :qn],
                            mybir.ActivationFunctionType.Identity,
                            bias=bt_s[:, 2 * g + ot:2 * g + ot + 1],
                            scale=1.0)
                        nc.sync.dma_start(
                            out=out_d[(2 * g + ot) * P:(2 * g + ot + 1) * P,
                                      q0c:q0c + qn],
                            in_=fo_s[(g, ot)][:, q0c:q0c + qn])

            # software pipeline: QK one step ahead of exp/PV so the PE
            # queue never head-of-line blocks on the ACT result
            emit_qk(steps[0])
            for i, step in enumerate(steps):
                c, h, kp = step
                q0 = c * qn
                if i + 1 < len(steps):
                    emit_qk(steps[i + 1])
                if kp == 0:
                    ps_o_cur[(c, h)] = ps_o_pool.tile(
                        [HD + 1, 512], f32, tag="ps_o", name="ps_o")
                ps_o = ps_o_cur[(c, h)]
                ps_s = ps_pend.pop(step)
                kcs = pairs[kp]
                w = len(kcs)
                e = e_pool.tile([P, 2, 512], bf16, tag="e", name="e")
                nc.scalar.activation(
                    e[:, :w, :qn], ps_s[:, :w, :qn],
                    mybir.ActivationFunctionType.Exp,
                    bias=0.0, scale=0.125)
                for idx, kc in enumerate(kcs):
                    nc.tensor.matmul(
                        ps_o[:, :qn],
                        va_t[:, kc, h, :],
                        e[:, idx, :qn],
                        start=(kc == 0), stop=(kc == nkc - 1))
                if kp == npair - 1:
                    rq = small.tile([1, 512], f32, tag="rq", name="rq")
                    nc.vector.reciprocal(rq[:, :qn], ps_o[HD:HD + 1, :qn])
                    sb_b = small.tile([HD, 512], f32, tag="sb_b",
                                      name="sb_b")
                    nc.gpsimd.partition_broadcast(sb_b[:, :qn], rq[:, :qn])
                    pq = h // 2
                    exc = (c == 1 and h >= 6)
                    if exc:
                        on = on_pool.tile([HD, 512], bf16, tag=f"on{h}",
                                          name=f"on{h}")
                        nc.vector.tensor_mul(on[:, :qn], ps_o[0:HD, :qn],
                                             sb_b[:, :qn])
                        on_head[(c, h)] = on
                    elif h % 2 == 0:
                        on2 = on_pool.tile([P, 512], bf16, tag=f"on2{pq}",
                                           name=f"on2{pq}")
                        on2_all[(c, pq)] = on2
                        nc.vector.tensor_mul(on2[0:HD, :qn], ps_o[0:HD, :qn],
                                             sb_b[:, :qn])
                    else:
                        # DVE writes in-lane; a plain SBUF->SBUF DMA shifts
                        # the odd head's output to partitions 64-127 so
                        # o_proj can contract over the full 128 rows
                        tmp = on_pool.tile([HD, 512], bf16, tag="on_tmp",
                                           name="on_tmp")
                        nc.vector.tensor_mul(tmp[:, :qn], ps_o[0:HD, :qn],
                                             sb_b[:, :qn])
                        nc.gpsimd.dma_start(
                            out=on2_all[(c, pq)][HD:P, :qn],
                            in_=tmp[:, :qn])

            emit_oproj(0)
            emit_oproj(1)
    nc.compile()
    return nc


def _prep_core_inputs(c, sqp, skp, q_idx, k_idx, query, key, value,
                      o_weight, o_bias):
    """Build the per-core input map. q_idx/k_idx are the compressed
    (unmasked) row indices per batch."""
    b, s = c // 2, c % 2
    dsl = slice(s * DS, (s + 1) * DS)

    qi = q_idx[b]
    ki = k_idx[b]
    nq, nk = len(qi), len(ki)

    nkc = skp // P
    qt = np.zeros((DS, sqp), BF)
    qt[:, :nq] = query[b][qi][:, dsl].T
    qt = qt.reshape(4, P, sqp).transpose(1, 0, 2)        # [P, 4, sqp]
    kt = np.zeros((DS, skp), BF)
    kt[:, :nk] = key[b][ki][:, dsl].T
    kt = kt.reshape(4, P, skp).transpose(1, 0, 2)        # [P, 4, skp]
    va = np.zeros((skp, HPC, HD + 1), BF)
    va[:nk, :, :HD] = value[b][ki][:, dsl].reshape(nk, HPC, HD)
    va[:nk, :, HD] = 1.0
    va = va.reshape(nkc, P, HPC, HD + 1).transpose(1, 0, 2, 3)

    # wt[d, (g ot ic), m]: lhsT chunks of W[2s+g][ot*128+m, ic*64+d]
    wt = np.zeros((16, HD, P), BF)
    for g in range(2):
        wg = o_weight[2 * s + g]
        for ot in range(2):
            for ic in range(4):
                wt[(2 * g + ot) * 4 + ic] = wg[ot * P:(ot + 1) * P,
                                               ic * HD:(ic + 1) * HD].T
    wt2 = np.zeros((8, P, P), BF)
    for g in range(2):
        wg = o_weight[2 * s + g]
        for ot in range(2):
            for p in range(2):
                wt2[g * 4 + ot * 2 + p] = wg[ot * P:(ot + 1) * P,
                                             p * P:(p + 1) * P].T
    wt2 = np.ascontiguousarray(wt2.transpose(1, 0, 2))   # [P, 8, P]
    wt = wt.transpose(1, 0, 2)                           # [HD, 16, P]
    bt = np.ascontiguousarray(o_bias[dsl].reshape(4, P).T.astype(np.float32))
    return {"qt": np.ascontiguousarray(qt), "kt": np.ascontiguousarray(kt),
            "va": np.ascontiguousarray(va), "wt": np.ascontiguousarray(wt),
            "wt2": wt2,
            "bt": bt}


def kernel(query, key, value, key_mask, query_mask, o_weight, o_bias):
    query = np.asarray(query, np.float32)
    key = np.asarray(key, np.float32)
    value = np.asarray(value, np.float32)
    key_mask = np.asarray(key_mask)
    query_mask = np.asarray(query_mask)
    o_weight = np.asarray(o_weight, np.float32)
    o_bias = np.asarray(o_bias, np.float32)

    k_idx = [np.nonzero(key_mask[b, :, 0])[0] for b in range(B)]
    q_idx = [np.nonzero(query_mask[b, :, 0])[0] for b in range(B)]
    skp = max(P, _pad_up(max(len(i) for i in k_idx), P))
    sqp = max(256, _pad_up(max(len(i) for i in q_idx), 4))

    if (sqp, skp) not in _CACHE:
        _CACHE[(sqp, skp)] = build_nc(sqp, skp)
    nc = _CACHE[(sqp, skp)]

    in_maps = [
        _prep_core_inputs(c, sqp, skp, q_idx, k_idx, query, key, value,
                          o_weight, o_bias)
        for c in range(NCORE)
    ]
    res = run_bass_kernel_spmd(nc, in_maps, core_ids=list(range(NCORE)),
                               trace=TRACE)
    LAST_RUN["exec_time_ns"] = res.exec_time_ns
    LAST_RUN["profile_json"] = res.profile_json
    LAST_RUN["results"] = res

    out = np.empty((B, SQ, D), np.float32)
    for c in range(NCORE):
        b, s = c // 2, c % 2
        core_out = res.results[c]["out"]              # [DS, sqp]
        qi = q_idx[b]
        out[b, :, s * DS:(s + 1) * DS] = o_bias[s * DS:(s + 1) * DS]
        out[b, qi, s * DS:(s + 1) * DS] = core_out[:, :len(qi)].T
    return out
